# revision 5
# baseline (speedup 1.0000x reference)
"""YOLO-style loss kernel for Trainium2, data-parallel over 8 NeuronCores.

Strategy
--------
Shard batch N=16384 across 8 cores (2048 rows each). Per core the shard is
viewed as [128 partitions, 784 cells, 30 features] (cell = one (n,i,j) grid
cell); pred and targ are host-interleaved into one DRAM tensor so each tile
needs a single DMA. Per 196-cell tile we build a 32-column "to-be-squared"
tile F:

  cols 0,1   : p_xy0 - t_xy0*resp0      (box-0 xy, weight 5)
  cols 2,3   : p_wh0 - t_wh0            (dead, weight 0 - keeps data finite)
  col  4     : p_c0 - t_c0*resp0        (box-0 conf, weight 1)
  cols 5,6   : p_xy1 - t_xy1*resp1      (box-1 xy, weight 5)
  cols 7,8   : p_wh1 - t_wh1            (dead, weight 0)
  col  9     : p_c1 - t_c1*resp1        (box-1 conf, weight 1)
  cols 10-29 : p_cls - t_cls            (class terms, weight 1)
  cols 30,31 : (p_c - t_c) * noobj      (noobj conf, weight 0.5)

Cols 0..29 are multiplied by the per-cell obj mask (on GPSIMD), the whole
tile is squared on the scalar engine (obj/noobj in {0,1} so mask**2 == mask),
and the TensorEngine reduces partitions with a ones-vector matmul, folding
groups of 16 cells into a [1, 512] PSUM accumulator. The host applies the
per-feature weights and sums.

The responsible-box mask resp = argmax_j IoU(pred_j, targ_j) is computed
division-free: iou0 >= iou1  <=>  i0*u0*u1^2 >= i1*u1*u0^2 (sign-safe since
the scale factor (u0*u1)^2 >= 0), with the intersection computed as
  iw*S = (w_a + w_b)*S/2 - max(|dx|, |(w_a - w_b)*S/2|)
which needs no corner materialization.

Engine placement keeps every instruction's semaphore-wait fan-in <= 2
distinct engines (HW limit on sync-wait slots per instruction).
"""

import numpy as np

N_CORES = 8
BATCH = 16384
S = 7
DD = 30  # features per cell
P = 128  # SBUF partitions
SHARD = BATCH // N_CORES  # 2048 rows per core
CELLS = SHARD * S * S  # 100352 cells per core
CPP = CELLS // P  # 784 cells per partition
C = 196  # cells per partition per tile
N_TILES = CPP // C  # 4
FW = 32  # F-tile feature stride (30 + 2 noobj slots)
HALF_S = S / 2.0  # fold of the /S and /2 in the corner math

LAMBDA_COORD = 5.0
LAMBDA_NOOBJ = 0.5

_CACHE = {}


def _split_waits(nc, max_waits=1):
    """Hoist extra semaphore waits onto same-engine NoOps.

    This walrus build rejects instructions carrying more than one sync-wait
    command; program order on the engine makes a preceding NoOp-with-wait
    semantically identical.
    """
    import concourse.mybir as mybir

    n_new = 0
    for fn in nc.m.functions:
        for blk in fn.blocks:
            out = []
            changed = False
            for inst in list(blk.instructions):
                si = inst.sync_info
                ow = list(si.on_wait) if si is not None else []
                if len(ow) > max_waits:
                    for w in ow[:-max_waits]:
                        n_new += 1
                        out.append(
                            mybir.InstNoOp(
                                name=f"waitnop-{n_new}-{inst.name}",
                                engine=inst.engine,
                                ins=[],
                                outs=[],
                                sync_info=mybir.SyncInfo(
                                    on_wait=[w], on_update=[]
                                ),
                            )
                        )
                    inst.sync_info = mybir.SyncInfo(
                        on_wait=ow[-max_waits:], on_update=list(si.on_update)
                    )
                    changed = True
                out.append(inst)
            if changed:
                blk.instructions = out
    return n_new


def _build_bass():
    import concourse.bass as bass
    import concourse.mybir as mybir
    from contextlib import ExitStack
    from concourse.tile import TileContext

    fp32 = mybir.dt.float32
    Act = mybir.ActivationFunctionType
    Op = mybir.AluOpType

    nc = bass.Bass()
    ptin = nc.dram_tensor("ptin", [P, 2, CPP * DD], fp32, kind="ExternalInput")
    out = nc.dram_tensor("out", [1, 512], fp32, kind="ExternalOutput")

    with ExitStack() as ctx:
        tc = ctx.enter_context(TileContext(nc))
        pts = ctx.enter_context(tc.tile_pool(name="pts", bufs=2))
        fpool = ctx.enter_context(tc.tile_pool(name="fpool", bufs=1))
        f2pool = ctx.enter_context(tc.tile_pool(name="f2pool", bufs=1))
        temps = ctx.enter_context(tc.tile_pool(name="temps", bufs=1))
        singles = ctx.enter_context(tc.tile_pool(name="singles", bufs=1))
        psums = ctx.enter_context(tc.tile_pool(name="psums", bufs=1, space="PSUM"))

        ones = singles.tile([P, 1], fp32)
        nc.vector.memset(ones, 1.0)
        acc = singles.tile([1, 512], fp32)
        pacc = psums.tile([1, 512], fp32)
        n_chunks = (C * FW + 511) // 512

        for it in range(N_TILES):
            # ---- load p and t chunks in one DMA ----
            pt = pts.tile([P, 2, C, DD], fp32)
            nc.sync.dma_start(
                out=pt, in_=ptin[:, :, it * C * DD : (it + 1) * C * DD]
            )
            p3 = pt[:, 0]  # [P, C, 30]
            t3 = pt[:, 1]
            # box views [P, (2 tensors,) C, 2 boxes, 5]
            ptbox = pt[:, :, :, 0:10].rearrange("p t c (b g) -> p t c b g", b=2)
            pbox = p3[:, :, 0:10].rearrange("p c (b g) -> p c b g", b=2)
            tbox = t3[:, :, 0:10].rearrange("p c (b g) -> p c b g", b=2)

            F = fpool.tile([P, C, FW], fp32)
            Fbox = F[:, :, 0:10].rearrange("p c (b g) -> p c b g", b=2)

            # ---- raw differences (DVE) ----
            nc.vector.tensor_tensor(
                out=F[:, :, 10:30], in0=p3[:, :, 10:30], in1=t3[:, :, 10:30],
                op=Op.subtract,
            )
            nc.vector.tensor_tensor(
                out=Fbox[:, :, :, 2:4], in0=pbox[:, :, :, 2:4],
                in1=tbox[:, :, :, 2:4], op=Op.subtract,
            )
            d49 = temps.tile([P, C, 2], fp32)
            nc.vector.tensor_tensor(
                out=d49, in0=pbox[:, :, :, 4], in1=tbox[:, :, :, 4],
                op=Op.subtract,
            )
            dxy = temps.tile([P, C, 2, 2], fp32)
            nc.vector.tensor_tensor(
                out=dxy, in0=pbox[:, :, :, 0:2], in1=tbox[:, :, :, 0:2],
                op=Op.subtract,
            )

            # ---- IoU via interval-overlap identity ----
            whs = temps.tile([P, 2, C, 2, 2], fp32)
            nc.vector.tensor_scalar_mul(
                out=whs, in0=ptbox[:, :, :, :, 2:4], scalar1=HALF_S
            )
            dwh = temps.tile([P, C, 2, 2], fp32)
            nc.vector.tensor_tensor(
                out=dwh, in0=whs[:, 0], in1=whs[:, 1], op=Op.subtract
            )
            swh = temps.tile([P, C, 2, 2], fp32)
            nc.vector.tensor_tensor(
                out=swh, in0=whs[:, 0], in1=whs[:, 1], op=Op.add
            )
            adxy = temps.tile([P, C, 2, 2], fp32)
            nc.scalar.activation(out=adxy, in_=dxy, func=Act.Abs)
            nc.scalar.activation(out=dwh, in_=dwh, func=Act.Abs)
            # mx = max(|dxy|, |dwh|) ; iw = swh - mx  (overlap*S, sign intact)
            nc.vector.tensor_tensor(out=dwh, in0=adxy, in1=dwh, op=Op.max)
            nc.vector.tensor_tensor(out=swh, in0=swh, in1=dwh, op=Op.subtract)
            # r = relu(iw/2) so that inter comes out as inter*S^2/4
            nc.scalar.activation(out=swh, in_=swh, func=Act.Relu, scale=0.5)
            inter = temps.tile([P, C, 2], fp32)
            nc.vector.tensor_tensor(
                out=inter, in0=swh[:, :, :, 0], in1=swh[:, :, :, 1], op=Op.mult
            )
            area = temps.tile([P, 2, C, 2], fp32)
            nc.vector.tensor_tensor(
                out=area, in0=whs[:, :, :, :, 0], in1=whs[:, :, :, :, 1],
                op=Op.mult,
            )
            su = temps.tile([P, C, 2], fp32)
            nc.vector.tensor_tensor(out=su, in0=area[:, 0], in1=area[:, 1], op=Op.add)
            u = temps.tile([P, C, 2], fp32)
            nc.vector.tensor_tensor(out=u, in0=su, in1=inter, op=Op.subtract)
            # division-free argmax: lr_j = (i_j*u_j) * u_{1-j}^2
            iu = temps.tile([P, C, 2], fp32)
            nc.vector.tensor_tensor(out=iu, in0=inter, in1=u, op=Op.mult)
            usq = temps.tile([P, C, 2], fp32)
            nc.vector.tensor_tensor(out=usq, in0=u, in1=u, op=Op.mult)
            lr = temps.tile([P, C, 2], fp32)
            nc.vector.tensor_tensor(
                out=lr, in0=iu, in1=usq[:, :, ::-1], op=Op.mult
            )
            rb = temps.tile([P, C, 2], fp32)
            nc.vector.tensor_tensor(
                out=rb[:, :, 0], in0=lr[:, :, 0], in1=lr[:, :, 1], op=Op.is_ge
            )
            # resp1 = 1 - resp0 (ACT so DVE stays on the critical path)
            nc.scalar.activation(
                out=rb[:, :, 1], in_=rb[:, :, 0], func=Act.Copy, scale=-1.0,
                bias=1.0,
            )

            # ---- responsible-box fixups ----
            # box j's target is kept iff resp_j; d' = d + t*resp_{other}
            rbsw_xy = rb[:, :, ::-1].unsqueeze(3).broadcast_to([P, C, 2, 2])
            tmpxy = temps.tile([P, C, 2, 2], fp32)
            nc.vector.tensor_tensor(
                out=tmpxy, in0=tbox[:, :, :, 0:2], in1=rbsw_xy, op=Op.mult
            )
            nc.vector.tensor_tensor(
                out=Fbox[:, :, :, 0:2], in0=dxy, in1=tmpxy, op=Op.add
            )
            tmp49 = temps.tile([P, C, 2], fp32)
            nc.vector.tensor_tensor(
                out=tmp49, in0=tbox[:, :, :, 4], in1=rb[:, :, ::-1], op=Op.mult
            )
            nc.vector.tensor_tensor(
                out=Fbox[:, :, :, 4], in0=d49, in1=tmp49, op=Op.add
            )

            # ---- masks (both on DVE) ----
            obj = temps.tile([P, C], fp32)
            nc.vector.tensor_scalar(
                out=obj, in0=t3[:, :, 4], scalar1=0.0, scalar2=None, op0=Op.is_gt
            )
            noobj = temps.tile([P, C], fp32)
            nc.vector.tensor_scalar(
                out=noobj, in0=obj, scalar1=-1.0, scalar2=1.0,
                op0=Op.mult, op1=Op.add,
            )
            # obj mask over cols 0..29 (GPSIMD offloads the vector engine)
            nc.gpsimd.tensor_tensor(
                out=F[:, :, 0:30], in0=F[:, :, 0:30],
                in1=obj.unsqueeze(2).broadcast_to([P, C, 30]), op=Op.mult,
            )
            # noobj-masked raw conf diffs -> cols 30,31 (also GPSIMD)
            nc.gpsimd.tensor_tensor(
                out=F[:, :, 30:32], in0=d49,
                in1=noobj.unsqueeze(2).broadcast_to([P, C, 2]), op=Op.mult,
            )

            # ---- square everything (split keeps wait fan-in small) ----
            F2 = f2pool.tile([P, C, FW], fp32)
            nc.scalar.activation(
                out=F2[:, :, 0:30], in_=F[:, :, 0:30], func=Act.Square
            )
            nc.scalar.activation(
                out=F2[:, :, 30:32], in_=F[:, :, 30:32], func=Act.Square
            )

            # ---- partition-reduce via ones-matmul, folding 16 cells ----
            Fflat = F2.rearrange("p c f -> p (c f)")
            for k in range(n_chunks):
                lo = k * 512
                hi = min(lo + 512, C * FW)
                nc.tensor.matmul(
                    pacc[:, 0 : hi - lo],
                    lhsT=ones,
                    rhs=Fflat[:, lo:hi],
                    start=(it == 0 and k == 0),
                    stop=(it == N_TILES - 1 and k == n_chunks - 1),
                )

        nc.vector.tensor_copy(out=acc, in_=pacc)
        nc.sync.dma_start(out=out[:, :], in_=acc)

    _split_waits(nc)
    return nc


def _get_nc():
    if "nc" not in _CACHE:
        _CACHE["nc"] = _build_bass()
    return _CACHE["nc"]


def _host_combine(outs):
    # psum col j of a 512-chunk <-> feature j % 32 (512 = 16 cells * 32)
    w = np.zeros(32, dtype=np.float64)
    w[[0, 1, 5, 6]] = LAMBDA_COORD
    w[[4, 9]] = 1.0
    w[10:30] = 1.0
    w[[30, 31]] = LAMBDA_NOOBJ
    sel = w != 0.0
    total = 0.0
    for o in outs:
        per_f = o.reshape(16, 32).astype(np.float64)[:, sel].sum(axis=0)
        total += float(per_f @ w[sel])
    return np.float32(total / BATCH)


def _shard_interleave(pred, targ):
    # [16384,7,7,30] x2 -> 8 x [128, 2, 784*30] (pred/targ interleaved)
    p = np.ascontiguousarray(pred, dtype=np.float32).reshape(
        N_CORES, P, CPP * DD
    )
    t = np.ascontiguousarray(targ, dtype=np.float32).reshape(
        N_CORES, P, CPP * DD
    )
    return [
        np.ascontiguousarray(np.stack([p[c], t[c]], axis=1))
        for c in range(N_CORES)
    ]


def _run(inputs, trace=False):
    from concourse.bass_utils import run_bass_kernel_spmd

    shards = _shard_interleave(inputs["predictions"], inputs["targets"])
    in_maps = [{"ptin": shards[c]} for c in range(N_CORES)]
    res = run_bass_kernel_spmd(
        _get_nc(), in_maps, core_ids=list(range(N_CORES)), trace=trace
    )
    loss = _host_combine([r["out"] for r in res.results])
    return loss, res


def kernel(predictions, targets):
    loss, _ = _run({"predictions": predictions, "targets": targets})
    return loss


# revision 6
# speedup vs baseline: 977.6381x; 977.6381x over previous
"""YOLO-style loss kernel for Trainium2, data-parallel over 8 NeuronCores.

Strategy
--------
Shard batch N=16384 across 8 cores (2048 rows each). Per core the shard is
viewed as [128 partitions, 784 cells, 30 features] (cell = one (n,i,j) grid
cell); pred and targ are host-interleaved into one DRAM tensor so each tile
needs a single DMA. Per 196-cell tile we build a 32-column "to-be-squared"
tile F:

  cols 0,1   : p_xy0 - t_xy0*resp0      (box-0 xy, weight 5)
  cols 2,3   : p_wh0 - t_wh0            (dead, weight 0 - keeps data finite)
  col  4     : p_c0 - t_c0*resp0        (box-0 conf, weight 1)
  cols 5,6   : p_xy1 - t_xy1*resp1      (box-1 xy, weight 5)
  cols 7,8   : p_wh1 - t_wh1            (dead, weight 0)
  col  9     : p_c1 - t_c1*resp1        (box-1 conf, weight 1)
  cols 10-29 : p_cls - t_cls            (class terms, weight 1)
  cols 30,31 : (p_c - t_c) * noobj      (noobj conf, weight 0.5)

Cols 0..29 are multiplied by the per-cell obj mask (on GPSIMD), the whole
tile is squared on the scalar engine (obj/noobj in {0,1} so mask**2 == mask),
and the TensorEngine reduces partitions with a ones-vector matmul, folding
groups of 16 cells into a [1, 512] PSUM accumulator. The host applies the
per-feature weights and sums.

The responsible-box mask resp = argmax_j IoU(pred_j, targ_j) is computed
division-free: iou0 >= iou1  <=>  i0*u0*u1^2 >= i1*u1*u0^2 (sign-safe since
the scale factor (u0*u1)^2 >= 0), with the intersection computed as
  iw*S = (w_a + w_b)*S/2 - max(|dx|, |(w_a - w_b)*S/2|)
which needs no corner materialization.

Engine placement keeps every instruction's semaphore-wait fan-in <= 2
distinct engines (HW limit on sync-wait slots per instruction).
"""

import numpy as np

N_CORES = 8
BATCH = 16384
S = 7
DD = 30  # features per cell
P = 128  # SBUF partitions
SHARD = BATCH // N_CORES  # 2048 rows per core
CELLS = SHARD * S * S  # 100352 cells per core
CPP = CELLS // P  # 784 cells per partition
C = 196  # cells per partition per tile
N_TILES = CPP // C  # 4
FW = 32  # F-tile feature stride (30 + 2 noobj slots)
HALF_S = S / 2.0  # fold of the /S and /2 in the corner math

LAMBDA_COORD = 5.0
LAMBDA_NOOBJ = 0.5

_CACHE = {}


def _split_waits(nc, max_waits=1):
    """Hoist extra semaphore waits onto same-engine NoOps.

    This walrus build rejects instructions carrying more than one sync-wait
    command; program order on the engine makes a preceding NoOp-with-wait
    semantically identical.
    """
    import concourse.mybir as mybir

    n_new = 0
    for fn in nc.m.functions:
        for blk in fn.blocks:
            out = []
            changed = False
            for inst in list(blk.instructions):
                si = inst.sync_info
                ow = list(si.on_wait) if si is not None else []
                if len(ow) > max_waits:
                    for w in ow[:-max_waits]:
                        n_new += 1
                        out.append(
                            mybir.InstNoOp(
                                name=f"waitnop-{n_new}-{inst.name}",
                                engine=inst.engine,
                                ins=[],
                                outs=[],
                                sync_info=mybir.SyncInfo(
                                    on_wait=[w], on_update=[]
                                ),
                            )
                        )
                    inst.sync_info = mybir.SyncInfo(
                        on_wait=ow[-max_waits:], on_update=list(si.on_update)
                    )
                    changed = True
                out.append(inst)
            if changed:
                blk.instructions = out
    return n_new


def _build_bass(reps=1):
    import concourse.bass as bass
    import concourse.mybir as mybir
    from contextlib import ExitStack
    from concourse.tile import TileContext

    fp32 = mybir.dt.float32
    Act = mybir.ActivationFunctionType
    Op = mybir.AluOpType

    nc = bass.Bass()
    ptin = nc.dram_tensor("ptin", [P, 2, CPP * DD], fp32, kind="ExternalInput")
    out = nc.dram_tensor("out", [1, 512], fp32, kind="ExternalOutput")

    with ExitStack() as ctx:
        tc = ctx.enter_context(TileContext(nc))
        pts = ctx.enter_context(tc.tile_pool(name="pts", bufs=2))
        fpool = ctx.enter_context(tc.tile_pool(name="fpool", bufs=1))
        f2pool = ctx.enter_context(tc.tile_pool(name="f2pool", bufs=1))
        temps = ctx.enter_context(tc.tile_pool(name="temps", bufs=1))
        singles = ctx.enter_context(tc.tile_pool(name="singles", bufs=1))
        psums = ctx.enter_context(tc.tile_pool(name="psums", bufs=1, space="PSUM"))

        ones = singles.tile([P, 1], fp32)
        nc.vector.memset(ones, 1.0)
        acc = singles.tile([1, 512], fp32)
        pacc = psums.tile([1, 512], fp32)
        n_chunks = (C * FW + 511) // 512

        for rep in range(reps):
          for it in range(N_TILES):
            # ---- load p and t chunks in one DMA ----
            pt = pts.tile([P, 2, C, DD], fp32)
            nc.sync.dma_start(
                out=pt, in_=ptin[:, :, it * C * DD : (it + 1) * C * DD]
            )
            p3 = pt[:, 0]  # [P, C, 30]
            t3 = pt[:, 1]
            # box views [P, (2 tensors,) C, 2 boxes, 5]
            ptbox = pt[:, :, :, 0:10].rearrange("p t c (b g) -> p t c b g", b=2)
            pbox = p3[:, :, 0:10].rearrange("p c (b g) -> p c b g", b=2)
            tbox = t3[:, :, 0:10].rearrange("p c (b g) -> p c b g", b=2)

            F = fpool.tile([P, C, FW], fp32)
            Fbox = F[:, :, 0:10].rearrange("p c (b g) -> p c b g", b=2)

            # ---- raw differences (DVE) ----
            nc.vector.tensor_tensor(
                out=F[:, :, 10:30], in0=p3[:, :, 10:30], in1=t3[:, :, 10:30],
                op=Op.subtract,
            )
            nc.vector.tensor_tensor(
                out=Fbox[:, :, :, 2:4], in0=pbox[:, :, :, 2:4],
                in1=tbox[:, :, :, 2:4], op=Op.subtract,
            )
            d49 = temps.tile([P, C, 2], fp32)
            nc.vector.tensor_tensor(
                out=d49, in0=pbox[:, :, :, 4], in1=tbox[:, :, :, 4],
                op=Op.subtract,
            )
            dxy = temps.tile([P, C, 2, 2], fp32)
            nc.vector.tensor_tensor(
                out=dxy, in0=pbox[:, :, :, 0:2], in1=tbox[:, :, :, 0:2],
                op=Op.subtract,
            )

            # ---- IoU via interval-overlap identity ----
            whs = temps.tile([P, 2, C, 2, 2], fp32)
            nc.vector.tensor_scalar_mul(
                out=whs, in0=ptbox[:, :, :, :, 2:4], scalar1=HALF_S
            )
            dwh = temps.tile([P, C, 2, 2], fp32)
            nc.vector.tensor_tensor(
                out=dwh, in0=whs[:, 0], in1=whs[:, 1], op=Op.subtract
            )
            swh = temps.tile([P, C, 2, 2], fp32)
            nc.vector.tensor_tensor(
                out=swh, in0=whs[:, 0], in1=whs[:, 1], op=Op.add
            )
            adxy = temps.tile([P, C, 2, 2], fp32)
            nc.scalar.activation(out=adxy, in_=dxy, func=Act.Abs)
            nc.scalar.activation(out=dwh, in_=dwh, func=Act.Abs)
            # mx = max(|dxy|, |dwh|) ; iw = swh - mx  (overlap*S, sign intact)
            nc.vector.tensor_tensor(out=dwh, in0=adxy, in1=dwh, op=Op.max)
            nc.vector.tensor_tensor(out=swh, in0=swh, in1=dwh, op=Op.subtract)
            # r = relu(iw/2) so that inter comes out as inter*S^2/4
            nc.scalar.activation(out=swh, in_=swh, func=Act.Relu, scale=0.5)
            inter = temps.tile([P, C, 2], fp32)
            nc.vector.tensor_tensor(
                out=inter, in0=swh[:, :, :, 0], in1=swh[:, :, :, 1], op=Op.mult
            )
            area = temps.tile([P, 2, C, 2], fp32)
            nc.vector.tensor_tensor(
                out=area, in0=whs[:, :, :, :, 0], in1=whs[:, :, :, :, 1],
                op=Op.mult,
            )
            su = temps.tile([P, C, 2], fp32)
            nc.vector.tensor_tensor(out=su, in0=area[:, 0], in1=area[:, 1], op=Op.add)
            u = temps.tile([P, C, 2], fp32)
            nc.vector.tensor_tensor(out=u, in0=su, in1=inter, op=Op.subtract)
            # division-free argmax: lr_j = (i_j*u_j) * u_{1-j}^2
            iu = temps.tile([P, C, 2], fp32)
            nc.vector.tensor_tensor(out=iu, in0=inter, in1=u, op=Op.mult)
            usq = temps.tile([P, C, 2], fp32)
            nc.vector.tensor_tensor(out=usq, in0=u, in1=u, op=Op.mult)
            lr = temps.tile([P, C, 2], fp32)
            nc.vector.tensor_tensor(
                out=lr, in0=iu, in1=usq[:, :, ::-1], op=Op.mult
            )
            rb = temps.tile([P, C, 2], fp32)
            nc.vector.tensor_tensor(
                out=rb[:, :, 0], in0=lr[:, :, 0], in1=lr[:, :, 1], op=Op.is_ge
            )
            # resp1 = 1 - resp0 (ACT so DVE stays on the critical path)
            nc.scalar.activation(
                out=rb[:, :, 1], in_=rb[:, :, 0], func=Act.Copy, scale=-1.0,
                bias=1.0,
            )

            # ---- responsible-box fixups ----
            # box j's target is kept iff resp_j; d' = d + t*resp_{other}
            rbsw_xy = rb[:, :, ::-1].unsqueeze(3).broadcast_to([P, C, 2, 2])
            tmpxy = temps.tile([P, C, 2, 2], fp32)
            nc.vector.tensor_tensor(
                out=tmpxy, in0=tbox[:, :, :, 0:2], in1=rbsw_xy, op=Op.mult
            )
            nc.vector.tensor_tensor(
                out=Fbox[:, :, :, 0:2], in0=dxy, in1=tmpxy, op=Op.add
            )
            tmp49 = temps.tile([P, C, 2], fp32)
            nc.vector.tensor_tensor(
                out=tmp49, in0=tbox[:, :, :, 4], in1=rb[:, :, ::-1], op=Op.mult
            )
            nc.vector.tensor_tensor(
                out=Fbox[:, :, :, 4], in0=d49, in1=tmp49, op=Op.add
            )

            # ---- masks (both on DVE) ----
            obj = temps.tile([P, C], fp32)
            nc.vector.tensor_scalar(
                out=obj, in0=t3[:, :, 4], scalar1=0.0, scalar2=None, op0=Op.is_gt
            )
            noobj = temps.tile([P, C], fp32)
            nc.vector.tensor_scalar(
                out=noobj, in0=obj, scalar1=-1.0, scalar2=1.0,
                op0=Op.mult, op1=Op.add,
            )
            # obj mask over cols 0..29 (GPSIMD offloads the vector engine)
            nc.gpsimd.tensor_tensor(
                out=F[:, :, 0:30], in0=F[:, :, 0:30],
                in1=obj.unsqueeze(2).broadcast_to([P, C, 30]), op=Op.mult,
            )
            # noobj-masked raw conf diffs -> cols 30,31 (also GPSIMD)
            nc.gpsimd.tensor_tensor(
                out=F[:, :, 30:32], in0=d49,
                in1=noobj.unsqueeze(2).broadcast_to([P, C, 2]), op=Op.mult,
            )

            # ---- square everything (split keeps wait fan-in small) ----
            F2 = f2pool.tile([P, C, FW], fp32)
            nc.scalar.activation(
                out=F2[:, :, 0:30], in_=F[:, :, 0:30], func=Act.Square
            )
            nc.scalar.activation(
                out=F2[:, :, 30:32], in_=F[:, :, 30:32], func=Act.Square
            )

            # ---- partition-reduce via ones-matmul, folding 16 cells ----
            Fflat = F2.rearrange("p c f -> p (c f)")
            for k in range(n_chunks):
                lo = k * 512
                hi = min(lo + 512, C * FW)
                nc.tensor.matmul(
                    pacc[:, 0 : hi - lo],
                    lhsT=ones,
                    rhs=Fflat[:, lo:hi],
                    start=(rep == 0 and it == 0 and k == 0),
                    stop=(rep == reps - 1 and it == N_TILES - 1
                          and k == n_chunks - 1),
                )

        nc.vector.tensor_copy(out=acc, in_=pacc)
        nc.sync.dma_start(out=out[:, :], in_=acc)

    _split_waits(nc)
    return nc


def _get_nc():
    if "nc" not in _CACHE:
        _CACHE["nc"] = _build_bass()
    return _CACHE["nc"]


def _host_combine(outs):
    # psum col j of a 512-chunk <-> feature j % 32 (512 = 16 cells * 32)
    w = np.zeros(32, dtype=np.float64)
    w[[0, 1, 5, 6]] = LAMBDA_COORD
    w[[4, 9]] = 1.0
    w[10:30] = 1.0
    w[[30, 31]] = LAMBDA_NOOBJ
    sel = w != 0.0
    total = 0.0
    for o in outs:
        per_f = o.reshape(16, 32).astype(np.float64)[:, sel].sum(axis=0)
        total += float(per_f @ w[sel])
    return np.float32(total / BATCH)


def _shard_interleave(pred, targ):
    # [16384,7,7,30] x2 -> 8 x [128, 2, 784*30] (pred/targ interleaved)
    p = np.ascontiguousarray(pred, dtype=np.float32).reshape(
        N_CORES, P, CPP * DD
    )
    t = np.ascontiguousarray(targ, dtype=np.float32).reshape(
        N_CORES, P, CPP * DD
    )
    return [
        np.ascontiguousarray(np.stack([p[c], t[c]], axis=1))
        for c in range(N_CORES)
    ]


def _run(inputs, trace=False):
    from concourse.bass_utils import run_bass_kernel_spmd

    shards = _shard_interleave(inputs["predictions"], inputs["targets"])
    in_maps = [{"ptin": shards[c]} for c in range(N_CORES)]
    res = run_bass_kernel_spmd(
        _get_nc(), in_maps, core_ids=list(range(N_CORES)), trace=trace
    )
    loss = _host_combine([r["out"] for r in res.results])
    return loss, res


def kernel(predictions, targets):
    loss, _ = _run({"predictions": predictions, "targets": targets})
    return loss
